# revision 1
# baseline (speedup 1.0000x reference)
"""GATv2 link predictor on 8 TRN2 NeuronCores (Bass/Tile) — v2.

Key changes vs baseline (profiled at 5.13 ms):
- The dominant cost was dma_gather Q7 descriptor generation (~8.3 ns/index,
  2.36 ms busy on GpSimd).  Layer 1's per-edge gather is ELIMINATED: the
  host pre-gathers raw x rows into transposed slot order (pure data
  movement), the PE projects them per chunk (lhsT=x_slot^T, rhs=W1l) into
  the same PSUM group that adds the dst-side selection.  The numerator is
  aggregated as sum(ex * vs); the dst-side contribution is removed per
  NODE after aggregation (h = accV/den - xr_own + bias), which is exact.
- Layer 2 still needs one runtime gather (h is produced on-chip), but from
  a bf16 2-node-packed table (256B rows), bucketed by src parity.
- One-hot select/scatter masks are precomputed on the host in fp8 and
  streamed from DRAM; no is_equal / dstr broadcast on-chip.
- LeakyReLU via ACT Prelu straight out of PSUM; scatter accumulates in
  persistent PSUM across whole (pool, block) runs.
- AllGathers move bf16 hl / z only (6.4 MB instead of 25.6 + 12.8 f32).
- Decode gathers 256B packed z rows, pairs bucketed by (a%2, b%2).
"""

import math
import numpy as np
import ml_dtypes

N = 50000
E = 800000
EL = 100000
IN = 128
HID = 32
HEADS = 4
OUT = 64
NEG_SLOPE = 0.2
LN_EPS = 1e-5
NCORES = 8

TB = 8                       # chunks per tile (1024 edge slots)

F32 = np.float32
BF16 = ml_dtypes.bfloat16
FP8 = ml_dtypes.float8_e4m3


def _derive():
    global R, HALF, NB, RLAST
    R = N // NCORES
    HALF = N // 2
    NB = math.ceil(R / 128)
    RLAST = R - (NB - 1) * 128


_derive()

_CACHE = {}


def configure(n=50000, e=800000, el=100000, ncores=8):
    global N, E, EL, NCORES
    N, E, EL, NCORES = n, e, el, ncores
    _derive()
    assert R % 2 == 0 and N % 2 == 0
    _CACHE.clear()


# ----------------------------------------------------------------- host prep
def _wrap_idx(idx16):
    n = idx16.shape[0]
    return np.tile(idx16.reshape(n // 16, 16).T, (8, 1)).copy()


def build_edge_schedule(src, dst, npools, pool_of, idx_of, want_src=False):
    """Bucket edges by (pool, dst-block).  Returns shared sched plus
    per-rank: int16 gather idx (if idx_of), slot src ids (if want_src),
    and fp8 one-hot masks gc [q, chunk, c] / gt [c, chunk, q]."""
    rank = dst // R
    pool = pool_of(src)
    blk = (dst % R) // 128
    dstm = (dst % R) % 128

    counts = np.zeros((NCORES, npools, NB), np.int64)
    np.add.at(counts, (rank, pool, blk), 1)
    cmax = counts.max(axis=0)
    nchunks = (cmax + 127) // 128

    sched = []
    for p in range(npools):
        blocks = []
        for b in range(NB):
            blocks += [b] * int(nchunks[p, b])
        while len(blocks) % TB:
            blocks.append(NB - 1)
        sched.append(blocks)
    choff = np.cumsum([0] + [len(s) for s in sched])
    ncht = int(choff[-1])

    slot_of_block = []
    for p in range(npools):
        sob = {}
        for ci, b in enumerate(sched[p]):
            sob.setdefault(b, []).append(ci)
        slot_of_block.append(sob)

    order = np.lexsort((src, dst))
    src_s = src[order]
    rank_s, pool_s = rank[order], pool[order]
    blk_s, dstm_s = blk[order], dstm[order]
    idx_s = idx_of(src_s) if idx_of else None

    per_rank = []
    for r in range(NCORES):
        m = rank_s == r
        rsrc, rpool = src_s[m], pool_s[m]
        rblk, rdstm = blk_s[m], dstm_s[m]
        ridx = idx_s[m] if idx_of else None
        idx_full, srcs_full = [], []
        q_all, ch_all, d_all = [], [], []
        for p in range(npools):
            n_slots = len(sched[p]) * 128
            s_idx = np.zeros(n_slots, np.int16)
            s_src = np.full(n_slots, -1, np.int64)
            mp = rpool == p
            psrc, pblk, pdstm = rsrc[mp], rblk[mp], rdstm[mp]
            pidx = ridx[mp] if idx_of else None
            for b in range(NB):
                mb = pblk == b
                nb_e = int(mb.sum())
                if nb_e == 0:
                    continue
                chunks = slot_of_block[p][b]
                base = np.concatenate(
                    [np.arange(ci * 128, ci * 128 + 128) for ci in chunks])
                tgt = base[:nb_e]
                if idx_of:
                    s_idx[tgt] = pidx[mb]
                s_src[tgt] = psrc[mb]
                q_all.append(tgt % 128)
                ch_all.append(tgt // 128 + choff[p])
                d_all.append(pdstm[mb].astype(np.int64))
            if idx_of:
                idx_full.append(_wrap_idx(s_idx))
            srcs_full.append(s_src)
        gc = np.zeros((128, ncht, 128), FP8)
        gt = np.zeros((128, ncht, 128), FP8)
        if q_all:
            q = np.concatenate(q_all)
            ch = np.concatenate(ch_all)
            d = np.concatenate(d_all)
            one = FP8(1.0)
            gc[q, ch, d] = one
            gt[d, ch, q] = one
        ent = {"gc": gc, "gt": gt}
        if idx_of:
            ent["idx"] = np.concatenate(idx_full, axis=1)
        if want_src:
            ent["srcs"] = np.concatenate(srcs_full)
        per_rank.append(ent)
    return {"sched": sched, "choff": choff, "nch": ncht,
            "per_rank": per_rank}


def build_decode_schedule(eli):
    """4 groups keyed (a%2, b%2); idx = v//2 into the packed z table."""
    npairs = EL // NCORES
    a_all, b_all = eli[0].astype(np.int64), eli[1].astype(np.int64)
    gmax = np.zeros(4, np.int64)
    parts = []
    for r in range(NCORES):
        a = a_all[r * npairs:(r + 1) * npairs]
        b = b_all[r * npairs:(r + 1) * npairs]
        g = (a % 2) * 2 + (b % 2)
        parts.append((a, b, g))
        for k in range(4):
            gmax[k] = max(gmax[k], int((g == k).sum()))
    gslots = [int(v + TB * 128 - 1) // (TB * 128) * (TB * 128) for v in gmax]
    per_rank = []
    for r in range(NCORES):
        a, b, g = parts[r]
        ia, ib = [], []
        perm = np.full(sum(gslots), -1, np.int64)
        off = 0
        for k in range(4):
            if gslots[k] == 0:
                continue
            mk = g == k
            nk = int(mk.sum())
            sa = np.zeros(gslots[k], np.int16)
            sb = np.zeros(gslots[k], np.int16)
            sa[:nk] = (a[mk] // 2).astype(np.int16)
            sb[:nk] = (b[mk] // 2).astype(np.int16)
            perm[off:off + nk] = np.nonzero(mk)[0]
            ia.append(_wrap_idx(sa))
            ib.append(_wrap_idx(sb))
            off += gslots[k]
        per_rank.append({"ia": np.concatenate(ia, axis=1),
                         "ib": np.concatenate(ib, axis=1), "perm": perm})
    return {"gslots": gslots, "per_rank": per_rank, "npairs": npairs}


def host_prep(inputs):
    ei = np.asarray(inputs["edge_index"])
    loops = np.arange(N, dtype=np.int64)
    src = np.concatenate([ei[0], loops]).astype(np.int64)
    dst = np.concatenate([ei[1], loops]).astype(np.int64)

    # L1: single pool; host pre-gathers x^T slot columns (no device gather)
    s1 = build_edge_schedule(
        src, dst, 1, lambda s: np.zeros_like(s), None, want_src=True)
    # L2: pools by src parity; device gathers packed rows src//2
    s2 = build_edge_schedule(
        src, dst, 2, lambda s: s % 2,
        lambda s: (s // 2).astype(np.int16))
    ds = build_decode_schedule(np.asarray(inputs["edge_label_index"]))

    x = np.asarray(inputs["x"], F32)
    x_tb = np.ascontiguousarray(x.T).astype(BF16)      # [IN, N]
    att1 = np.asarray(inputs["att1"], F32).reshape(-1)
    att2 = np.asarray(inputs["att2"], F32).reshape(-1)
    b1l = np.asarray(inputs["b1l"], F32)
    b1r = np.asarray(inputs["b1r"], F32)
    bias1 = np.asarray(inputs["bias1"], F32)

    shared = {
        "W1l": np.asarray(inputs["W1l"], F32).astype(BF16),
        "W1r": np.asarray(inputs["W1r"], F32).astype(BF16),
        "W2l": np.asarray(inputs["W2l"], F32).astype(BF16),
        "W2r": np.asarray(inputs["W2r"], F32).astype(BF16),
        # vs needs xr side to carry BOTH linear biases (b1l + b1r)
        "b1rl": (b1r + b1l).reshape(1, -1).astype(BF16),
        "b2l": np.asarray(inputs["b2l"], F32).reshape(1, -1).astype(BF16),
        "b2r": np.asarray(inputs["b2r"], F32).reshape(1, -1).astype(BF16),
        "attr1f": np.ascontiguousarray(
            np.tile(att1.astype(BF16).reshape(1, -1), (128, TB))),
        "attr2f": np.ascontiguousarray(
            np.tile(att2.astype(BF16).reshape(1, -1), (128, TB))),
        # post subtracts xr_own (which carries b1l+b1r) and re-adds b1l
        # plus the GAT output bias -> combo row
        "b1c": np.tile((bias1 + b1l).reshape(1, -1), (128, 1)).astype(F32),
        "g1": np.tile(np.asarray(inputs["g1"], F32).reshape(1, -1),
                      (128, 1)),
        "be1": np.tile(np.asarray(inputs["be1"], F32).reshape(1, -1),
                       (128, 1)),
        "bias2": np.tile(np.asarray(inputs["bias2"], F32).reshape(1, -1),
                         (128, 1)),
        "g2": np.tile(np.asarray(inputs["g2"], F32).reshape(1, -1),
                      (128, 1)),
        "be2": np.tile(np.asarray(inputs["be2"], F32).reshape(1, -1),
                       (128, 1)),
        "ident": np.eye(128, dtype=F32).astype(BF16),
        "ones1": np.ones((1, 128), BF16),
    }
    in_maps = []
    for r in range(NCORES):
        m = dict(shared)
        m["x_own_t"] = np.ascontiguousarray(x_tb[:, r * R:(r + 1) * R])
        pr1 = s1["per_rank"][r]
        srcs = pr1["srcs"]
        xsl = np.zeros((IN, len(srcs)), BF16)
        valid = srcs >= 0
        xsl[:, valid] = x_tb[:, srcs[valid]]
        m["xsl"] = xsl
        m["gc1"], m["gt1"] = pr1["gc"], pr1["gt"]
        pr2 = s2["per_rank"][r]
        m["idx2"] = pr2["idx"]
        m["gc2"], m["gt2"] = pr2["gc"], pr2["gt"]
        dr = ds["per_rank"][r]
        m["dia"], m["dib"] = dr["ia"], dr["ib"]
        in_maps.append(m)
    return {"s1": s1, "s2": s2, "ds": ds}, in_maps


# ------------------------------------------------------------- kernel builder
def build_kernel(meta):
    import concourse.bacc as bacc
    import concourse.bass as bass
    import concourse.mybir as mybir
    import concourse.tile as tile
    from contextlib import ExitStack

    dt = mybir.dt
    AF = mybir.ActivationFunctionType
    OP = mybir.AluOpType
    AX = mybir.AxisListType
    AP = bass.AP

    s1, s2, ds = meta["s1"], meta["s2"], meta["ds"]
    sched1, choff1, nch1 = s1["sched"], s1["choff"], s1["nch"]
    sched2, choff2, nch2 = s2["sched"], s2["choff"], s2["nch"]
    gslots = ds["gslots"]
    n_dec_slots = sum(gslots)
    dec_groups = [(k, gslots[k] // (TB * 128))
                  for k in range(4) if gslots[k]]

    nc = bacc.Bacc("TRN2", target_bir_lowering=False, debug=False,
                   num_devices=NCORES)

    def din(name, shape, d=dt.float32):
        return nc.dram_tensor(name, list(shape), d, kind="ExternalInput")

    x_own_t = din("x_own_t", [IN, R], dt.bfloat16)
    xsl_in = din("xsl", [IN, nch1 * 128], dt.bfloat16)
    W1l, W1r = din("W1l", [IN, IN], dt.bfloat16), din("W1r", [IN, IN],
                                                      dt.bfloat16)
    W2l, W2r = din("W2l", [IN, OUT], dt.bfloat16), din("W2r", [IN, OUT],
                                                       dt.bfloat16)
    b1rl_in = din("b1rl", [1, IN], dt.bfloat16)
    b2l_in = din("b2l", [1, OUT], dt.bfloat16)
    b2r_in = din("b2r", [1, OUT], dt.bfloat16)
    attr1f_in = din("attr1f", [128, TB * IN], dt.bfloat16)
    attr2f_in = din("attr2f", [128, TB * OUT], dt.bfloat16)
    ln_rows = {nm: din(nm, [128, w]) for nm, w in [
        ("b1c", IN), ("g1", IN), ("be1", IN),
        ("bias2", OUT), ("g2", OUT), ("be2", OUT)]}
    ident_in = din("ident", [128, 128], dt.bfloat16)
    ones1_in = din("ones1", [1, 128], dt.bfloat16)
    gc1_in = din("gc1", [128, nch1, 128], dt.float8e4)
    gt1_in = din("gt1", [128, nch1, 128], dt.float8e4)
    idx2_in = din("idx2", [128, nch2 * 8], dt.int16)
    gc2_in = din("gc2", [128, nch2, 128], dt.float8e4)
    gt2_in = din("gt2", [128, nch2, 128], dt.float8e4)
    dia_in = din("dia", [128, n_dec_slots // 16], dt.int16)
    dib_in = din("dib", [128, n_dec_slots // 16], dt.int16)

    out_dec = nc.dram_tensor("out_dec", [n_dec_slots], dt.float32,
                             kind="ExternalOutput")

    aghl = nc.dram_tensor("aghl", [R // 2, 128], dt.bfloat16)
    hl_tab = nc.dram_tensor("hl_tab", [N // 2, 128], dt.bfloat16,
                            addr_space="Shared")
    agz = nc.dram_tensor("agz", [R // 2, 128], dt.bfloat16)
    z_tab = nc.dram_tensor("z_tab", [N // 2, 128], dt.bfloat16,
                           addr_space="Shared")

    grp = list(range(NCORES))

    with tile.TileContext(nc) as tc, ExitStack() as stack, \
            nc.allow_low_precision(reason="bf16 logit path; 2e-2 tolerance"):
        pp = stack.enter_context(tc.tile_pool(name="persist", bufs=1))

        def pload(t, name):
            s = pp.tile(list(t.shape), t.dtype, name=name, tag=name)
            nc.sync.dma_start(s[:], t[:])
            return s

        ident = pload(ident_in, "identS")
        ones1 = pload(ones1_in, "ones1S")
        w1l, w1r = pload(W1l, "w1lS"), pload(W1r, "w1rS")
        w2l, w2r = pload(W2l, "w2lS"), pload(W2r, "w2rS")
        b1rl = pload(b1rl_in, "b1rlS")
        b2l_s, b2r_s = pload(b2l_in, "b2lS"), pload(b2r_in, "b2rS")
        attr1f = pload(attr1f_in, "attr1fS")
        attr2f = pload(attr2f_in, "attr2fS")
        rows = {nm: pload(t, f"r_{nm}") for nm, t in ln_rows.items()}
        idx2 = pload(idx2_in, "idx2S")
        dia = pload(dia_in, "diaS")
        dib = pload(dib_in, "dibS")

        xr_own = pp.tile([128, NB, IN], dt.bfloat16)
        hr_own = pp.tile([128, NB, OUT], dt.bfloat16)
        acc1 = pp.tile([128, NB, IN + HEADS], dt.float32)
        acc2 = pp.tile([128, NB, OUT + 2], dt.float32)
        h_bf = pp.tile([128, NB, IN], dt.bfloat16)
        z_bf = pp.tile([128, NB, OUT], dt.bfloat16)
        out_sb = pp.tile([128, n_dec_slots // 128], dt.float32)

        # ------- xr_own = x_own @ W1r + (b1r + b1l) ----------------------
        with tc.tile_pool(name="pA", bufs=3) as rp, \
                tc.tile_pool(name="pAp", bufs=2, space="PSUM") as ps:
            for i0 in range(0, NB, 4):
                nblk = min(4, NB - i0)
                lo = i0 * 128
                cnt = min(4 * 128, R - lo)
                xT = rp.tile([128, 4, 128], dt.bfloat16, tag="xT")
                if cnt < 4 * 128:
                    nc.vector.memset(xT[:], 0.0)
                nc.sync.dma_start(
                    xT[:].rearrange("p a b -> p (a b)")[:, 0:cnt],
                    x_own_t[:, lo:lo + cnt])
                ps_t = ps.tile([128, 4, IN], dt.float32, tag="ps")
                for b in range(nblk):
                    nc.tensor.matmul(ps_t[:, b, :], lhsT=xT[:, b, :],
                                     rhs=w1r[:], start=True, stop=False)
                    nc.tensor.matmul(ps_t[:, b, :], lhsT=ones1[:],
                                     rhs=b1rl[:], start=False, stop=True)
                nc.scalar.activation(
                    xr_own[:, i0:i0 + nblk, :].rearrange("p a b -> p (a b)"),
                    ps_t[:, 0:nblk, :].rearrange("p a b -> p (a b)"),
                    AF.Copy)

        # ---------------- edge aggregation ------------------------------
        def edge_phase(feat, H, scheds, choff, gcD, gtD, side_own, attrF,
                       accum, w_proj, src_is_gather, tabs, idx_sb, tslice,
                       on_final=None, pool_ebuf=False):
            npools = len(scheds)
            contrib = {}
            for p in range(npools):
                for b in set(scheds[p]):
                    contrib.setdefault(b, []).append(p)
            ch = feat // H
            with tc.tile_pool(name=f"pE{feat}", bufs=4) as rp, \
                    tc.tile_pool(name=f"pEv{feat}", bufs=3, space="PSUM") \
                    as psv, \
                    tc.tile_pool(name=f"pEo{feat}", bufs=2, space="PSUM") \
                    as pso:
                oacc = None
                xsl_big = None
                for p in range(npools):
                    sched = scheds[p]
                    nt = len(sched) // TB
                    for t in range(nt):
                        ch0 = int(choff[p]) + t * TB
                        joff = 0
                        if src_is_gather:
                            if t % 2 == 0:
                                ng = min(2 * TB, (nt - t) * TB)
                                xsl_big = rp.tile([128, 2 * TB, 128],
                                                  dt.bfloat16, tag="xslg")
                                nc.gpsimd.dma_gather(
                                    xsl_big[:, 0:ng, :], tabs[:, 0:128],
                                    idx_sb[:, ch0 * 8:(ch0 + ng) * 8],
                                    ng * 128, ng * 128, 128,
                                    elem_step=128, single_packet=False)
                            xsl_t = xsl_big
                            joff = (t % 2) * TB
                            c0, c1 = tslice[p]
                        else:
                            xsl_t = rp.tile([128, TB, 128], dt.bfloat16,
                                            tag="xslT")
                            nc.sync.dma_start(
                                xsl_t[:].rearrange("p a b -> p (a b)"),
                                tabs[:, ch0 * 128:(ch0 + TB) * 128])
                        gc_sb = rp.tile([128, TB, 128], dt.float8e4,
                                        tag="gc")
                        nc.sync.dma_start(gc_sb[:], gcD[:, ch0:ch0 + TB, :])
                        gt_sb = rp.tile([128, TB, 128], dt.float8e4,
                                        tag="gt")
                        nc.sync.dma_start(gt_sb[:], gtD[:, ch0:ch0 + TB, :])
                        vs = psv.tile([128, TB, feat], dt.float32, tag="vs")
                        for j in range(TB):
                            B = sched[t * TB + j]
                            if src_is_gather:
                                nc.tensor.matmul(
                                    vs[:, j, :], lhsT=ident[:],
                                    rhs=xsl_t[:, joff + j, c0:c1],
                                    start=True, stop=False)
                            else:
                                nc.tensor.matmul(
                                    vs[:, j, :], lhsT=xsl_t[:, j, :],
                                    rhs=w_proj[:],
                                    start=True, stop=False)
                            nc.tensor.matmul(
                                vs[:, j, :], lhsT=gt_sb[:, j, :],
                                rhs=side_own[:, B, 0:feat],
                                start=False, stop=True)
                        rv = rp.tile([128, TB, feat], dt.bfloat16, tag="rv")
                        if pool_ebuf:
                            vs_sb = rp.tile([128, TB, feat], dt.bfloat16,
                                            tag="vsb")
                        hh = TB // 2
                        for h2 in range(2):
                            sl = vs[:, h2 * hh:(h2 + 1) * hh, :].rearrange(
                                "p a b -> p (a b)")
                            nc.scalar.activation(
                                rv[:, h2 * hh:(h2 + 1) * hh, :].rearrange(
                                    "p a b -> p (a b)"),
                                sl, AF.Prelu, alpha=NEG_SLOPE)
                            if pool_ebuf:
                                nc.scalar.activation(
                                    vs_sb[:, h2 * hh:(h2 + 1) * hh, :]
                                    .rearrange("p a b -> p (a b)"),
                                    sl, AF.Copy)
                        lm = rp.tile([128, TB * H, ch], dt.bfloat16,
                                     tag="lm")
                        nc.vector.tensor_tensor(
                            lm[:].rearrange("p a b -> p (a b)"),
                            rv[:].rearrange("p a b -> p (a b)"),
                            attrF[:, 0:TB * feat], op=OP.mult)
                        t1 = rp.tile([128, TB * H, ch // 2], dt.bfloat16,
                                     tag="t1")
                        nc.vector.tensor_tensor(
                            t1[:], lm[:, :, 0:ch // 2],
                            lm[:, :, ch // 2:ch], op=OP.add)
                        lg = rp.tile([128, TB, H], dt.bfloat16, tag="lg")
                        nc.vector.tensor_reduce(
                            lg[:].rearrange("p a b -> p (a b)"), t1[:],
                            axis=AX.X, op=OP.add)
                        ebuf = rp.tile([128, TB, feat + H + (H % 2)],
                                       dt.bfloat16, tag="ebuf")
                        nc.scalar.activation(
                            ebuf[:, :, feat:feat + H],
                            lg[:], AF.Exp)
                        if pool_ebuf:
                            nc.gpsimd.tensor_tensor(
                                ebuf[:, :, 0:feat].rearrange(
                                    "p t (h c) -> p t h c", h=H),
                                vs_sb[:].rearrange(
                                    "p t (h c) -> p t h c", h=H),
                                ebuf[:, :, feat:feat + H, None]
                                .to_broadcast([128, TB, H, ch]),
                                op=OP.mult)
                        else:
                            nc.vector.tensor_tensor(
                                ebuf[:, :, 0:feat].rearrange(
                                    "p t (h c) -> p t h c", h=H),
                                vs[:].rearrange("p t (h c) -> p t h c",
                                                h=H),
                                ebuf[:, :, feat:feat + H, None]
                                .to_broadcast([128, TB, H, ch]),
                                op=OP.mult)
                        for j in range(TB):
                            ci = t * TB + j
                            B = sched[ci]
                            first = ci == 0 or sched[ci - 1] != B
                            last = (ci == len(sched) - 1 or
                                    sched[ci + 1] != B)
                            if first:
                                oacc = pso.tile([128, feat + H],
                                                dt.float32, tag="oacc")
                            nc.tensor.matmul(
                                oacc[:], lhsT=gc_sb[:, j, :],
                                rhs=ebuf[:, j, 0:feat + H],
                                start=first, stop=last,
                                skip_group_check=True)
                            if last:
                                if contrib[B][0] == p:
                                    nc.scalar.activation(
                                        accum[:, B, 0:feat + H], oacc[:],
                                        AF.Copy)
                                else:
                                    nc.vector.tensor_add(
                                        accum[:, B, 0:feat + H],
                                        accum[:, B, 0:feat + H], oacc[:])
                                if p == npools - 1 and on_final:
                                    on_final(B)

        # ---------------- post layer ------------------------------------
        def bcast_row(ap, nb_, w):
            return AP(ap.tensor, ap.offset,
                      [[ap.ap[0][0], 128], [0, nb_], [1, w]])

        def post_group(rp, accum, feat, nheads, side_own, combo_row, g_row,
                       be_row, elu, out_tile, b0, nb_):
                    a = accum[:, b0:b0 + nb_, :]
                    dn = rp.tile([128, nb_, nheads], dt.float32, tag="dn")
                    nc.vector.tensor_scalar(
                        dn[:], a[:, :, feat:feat + nheads], 1e-16, None,
                        op0=OP.add)
                    rcp = rp.tile([128, nb_, nheads], dt.float32, tag="rcp")
                    nc.vector.reciprocal(rcp[:], dn[:])
                    hv = accum[:, b0:b0 + nb_, 0:feat]
                    ch = feat // nheads
                    hv4 = hv.rearrange("p b (h c) -> p b h c", h=nheads)
                    nc.vector.tensor_tensor(
                        hv4, hv4,
                        rcp[:, :, :, None].to_broadcast(
                            [128, nb_, nheads, ch]), op=OP.mult)
                    # subtract the dst-side rows folded into vs
                    nc.vector.tensor_tensor(
                        hv, hv, side_own[:, b0:b0 + nb_, 0:feat],
                        op=OP.subtract)
                    nc.vector.tensor_tensor(
                        hv, hv, bcast_row(combo_row[:], nb_, feat),
                        op=OP.add)
                    mu = rp.tile([128, nb_], dt.float32, tag="mu")
                    nc.vector.tensor_reduce(mu[:], hv, axis=AX.X, op=OP.add)
                    nc.vector.tensor_scalar_mul(mu[:], mu[:], 1.0 / feat)
                    nc.vector.tensor_tensor(
                        hv, hv,
                        mu[:, :, None].to_broadcast([128, nb_, feat]),
                        op=OP.subtract)
                    sq = rp.tile([128, nb_, feat], dt.float32, tag="sq")
                    nc.vector.tensor_tensor(sq[:], hv, hv, op=OP.mult)
                    var = rp.tile([128, nb_], dt.float32, tag="var")
                    nc.vector.tensor_reduce(var[:], sq[:], axis=AX.X,
                                            op=OP.add)
                    nc.vector.tensor_scalar(var[:], var[:], 1.0 / feat,
                                            LN_EPS, op0=OP.mult, op1=OP.add)
                    lnv = rp.tile([128, nb_], dt.float32, tag="lnv")
                    nc.scalar.activation(lnv[:], var[:], AF.Ln)
                    rs = rp.tile([128, nb_], dt.float32, tag="rs")
                    nc.scalar.activation(rs[:], lnv[:], AF.Exp, scale=-0.5)
                    nc.vector.tensor_tensor(
                        hv, hv,
                        rs[:, :, None].to_broadcast([128, nb_, feat]),
                        op=OP.mult)
                    nc.vector.tensor_tensor(
                        hv, hv, bcast_row(g_row[:], nb_, feat), op=OP.mult)
                    nc.vector.tensor_tensor(
                        hv, hv, bcast_row(be_row[:], nb_, feat), op=OP.add)
                    if elu:
                        nr = rp.tile([128, nb_, feat], dt.float32, tag="nr")
                        nc.scalar.activation(nr[:], hv, AF.Relu, scale=-1.0)
                        ex0 = rp.tile([128, nb_, feat], dt.float32,
                                      tag="ex0")
                        nc.scalar.activation(ex0[:], nr[:], AF.Exp,
                                             scale=-1.0)
                        nc.scalar.activation(hv, hv, AF.Relu)
                        tmp = rp.tile([128, nb_, feat], dt.float32,
                                      tag="tmp")
                        nc.vector.tensor_add(tmp[:], hv, ex0[:])
                        nc.vector.tensor_scalar(
                            out_tile[:, b0:b0 + nb_, :], tmp[:], -1.0,
                            None, op0=OP.add)
                    else:
                        nc.vector.tensor_copy(out_tile[:, b0:b0 + nb_, :],
                                              hv)

        def pack_out(table, B, cnt, st):
            for par in range(2):
                nc.sync.dma_start(
                    AP(table[:].tensor, B * 64 * 128 + par * OUT,
                       [[128, cnt // 2], [1, OUT]]),
                    AP(st.tensor, st.offset + par * st.ap[0][0],
                       [[2 * st.ap[0][0], cnt // 2], [1, OUT]]))

        # ================= layer 1 =======================================
        RD = 7
        edge_phase(IN, HEADS, sched1, choff1, gc1_in, gt1_in, xr_own,
                   attr1f, acc1, w1l, False, xsl_in, None, None)
        with tc.tile_pool(name="pD", bufs=3) as rpD, \
                tc.tile_pool(name="pDp", bufs=1, space="PSUM") as psD, \
                tc.tile_pool(name="pC1", bufs=2) as rpC1:

            def projD_block(B):
                cnt = 128 if B < NB - 1 else RLAST
                hT_ps = psD.tile([128, 128], dt.bfloat16, tag="hT")
                nc.tensor.transpose(hT_ps[:], h_bf[:, B, :], ident[:])
                hT = rpD.tile([128, 128], dt.bfloat16, tag="hTs")
                nc.vector.tensor_copy(hT[:], hT_ps[:])
                pl = psD.tile([128, OUT], dt.float32, tag="pl")
                nc.tensor.matmul(pl[:], lhsT=hT[:], rhs=w2l[:], start=True,
                                 stop=False)
                nc.tensor.matmul(pl[:], lhsT=ones1[:], rhs=b2l_s[:],
                                 start=False, stop=True)
                stl = rpD.tile([128, OUT], dt.bfloat16, tag="stl")
                nc.scalar.activation(stl[:], pl[:], AF.Copy)
                pack_out(aghl, B, cnt, stl[:])
                pr_ = psD.tile([128, OUT], dt.float32, tag="pl")
                nc.tensor.matmul(pr_[:], lhsT=hT[:], rhs=w2r[:], start=True,
                                 stop=False)
                nc.tensor.matmul(pr_[:], lhsT=ones1[:], rhs=b2r_s[:],
                                 start=False, stop=True)
                nc.scalar.activation(hr_own[:, B, :], pr_[:], AF.Copy)

            st1 = {"grp": 0}

            def on_final1(B):
                while st1["grp"] * RD < NB:
                    b0 = st1["grp"] * RD
                    nb_ = min(RD, NB - b0)
                    if b0 + nb_ - 1 > B:
                        break
                    post_group(rpC1, acc1, IN, HEADS, xr_own, rows["b1c"],
                               rows["g1"], rows["be1"], True, h_bf, b0, nb_)
                    for Bb in range(b0, b0 + nb_):
                        projD_block(Bb)
                    st1["grp"] += 1

            on_final1(NB - 1)
        nc.gpsimd.collective_compute(
            "AllGather", OP.bypass, replica_groups=[grp],
            ins=[aghl[:]], outs=[hl_tab[:]])

        # ================= layer 2 =======================================
        with tc.tile_pool(name="pC2", bufs=2) as rpC2:
            st2 = {"grp": 0}

            def on_final2(B):
                while st2["grp"] * RD < NB:
                    b0 = st2["grp"] * RD
                    nb_ = min(RD, NB - b0)
                    if b0 + nb_ - 1 > B:
                        break
                    post_group(rpC2, acc2, OUT, 1, hr_own, rows["bias2"],
                               rows["g2"], rows["be2"], False, z_bf, b0,
                               nb_)
                    for Bb in range(b0, b0 + nb_):
                        cnt = 128 if Bb < NB - 1 else RLAST
                        pack_out(agz, Bb, cnt, z_bf[:, Bb, :])
                    st2["grp"] += 1

            edge_phase(OUT, 1, sched2, choff2, gc2_in, gt2_in, hr_own,
                       attr2f, acc2, None, True, hl_tab, idx2,
                       [(0, OUT), (OUT, 2 * OUT)])
            on_final2(NB - 1)
        nc.gpsimd.collective_compute(
            "AllGather", OP.bypass, replica_groups=[grp],
            ins=[agz[:]], outs=[z_tab[:]])

        # ================= decode ========================================
        with tc.tile_pool(name="pG", bufs=4) as rp:
            toff = 0
            for k, ntk in dec_groups:
                apar, bpar = (k >> 1) & 1, k & 1
                for tt in range(ntk):
                    t = toff + tt
                    za = rp.tile([128, TB, 128], dt.bfloat16, tag="za")
                    nc.gpsimd.dma_gather(
                        za[:], z_tab[:, 0:128],
                        dia[:, t * TB * 8:(t + 1) * TB * 8],
                        TB * 128, TB * 128, 128, elem_step=128,
                        single_packet=False)
                    zb = rp.tile([128, TB, 128], dt.bfloat16, tag="zb")
                    nc.gpsimd.dma_gather(
                        zb[:], z_tab[:, 0:128],
                        dib[:, t * TB * 8:(t + 1) * TB * 8],
                        TB * 128, TB * 128, 128, elem_step=128,
                        single_packet=False)
                    prod = rp.tile([128, TB, OUT], dt.float32, tag="prod")
                    nc.vector.tensor_tensor(
                        prod[:], za[:, :, apar * OUT:(apar + 1) * OUT],
                        zb[:, :, bpar * OUT:(bpar + 1) * OUT], op=OP.mult)
                    nc.vector.tensor_reduce(
                        out_sb[:, t * TB:(t + 1) * TB], prod[:],
                        axis=AX.X, op=OP.add)
                toff += ntk
            nc.sync.dma_start(
                AP(out_dec[:].tensor, 0,
                   [[1, 128], [128, n_dec_slots // 128]]),
                out_sb[:])

    nc.compile()
    return nc


# ------------------------------------------------------------------ runner
def kernel(_trace=False, **inputs):
    from concourse.bass_utils import run_bass_kernel_spmd

    meta, in_maps = host_prep(inputs)
    key = "k"
    if key not in _CACHE:
        _CACHE[key] = build_kernel(meta)
    nc = _CACHE[key]
    res = run_bass_kernel_spmd(nc, in_maps, list(range(NCORES)),
                               trace=bool(_trace))
    npairs = meta["ds"]["npairs"]
    out = np.zeros(EL, F32)
    for r in range(NCORES):
        od = res.results[r]["out_dec"]
        perm = meta["ds"]["per_rank"][r]["perm"]
        m = perm >= 0
        out[r * npairs + perm[m]] = od[m]
    if _trace:
        return out, res
    return out

